# revision 1
# baseline (speedup 1.0000x reference)
import sys

import numpy as np

sys.path.insert(0, "/opt/trn_rl_repo")

_B, _S, _T = 2048, 4096, 3
_NC = 8
_BL = _B // _NC  # 256 seqs per core
_P = 128
_G = _BL // _P  # 2 seqs per partition
_SLAB = 1024
_NSLAB = _S // _SLAB

# params tile layout (host-filled, broadcast to 128 partitions):
#   [0:27)   A2[(i,j),k] = A[i,k]*A[k,j]        (A = exp(transitions))
#   [27:36)  C0[(i,j)]   = sv[i]*A[i,j]         (sv = exp(start))
#   [36:45)  T9[3p+c]    = transitions[p,c]
#   [45:48)  st[j], [48:51) en[j]
#   [51:57)  ev tiled twice (exp(end), exp(end))
_PRW = 64

_cache = {}


def _build():
    from concourse import bacc, mybir
    from concourse.tile import TileContext

    f32 = mybir.dt.float32
    bf16 = mybir.dt.bfloat16
    i32 = mybir.dt.int32
    Alu = mybir.AluOpType
    Act = mybir.ActivationFunctionType
    Ax = mybir.AxisListType

    nc = bacc.Bacc("TRN2", target_bir_lowering=False)
    em_d = nc.dram_tensor("em", (_BL, _S, _T), f32, kind="ExternalInput")
    tg_d = nc.dram_tensor("tg", (_BL, _S), i32, kind="ExternalInput")
    pr_d = nc.dram_tensor("pr", (_P, _PRW), f32, kind="ExternalInput")
    out_d = nc.dram_tensor("out", (_P, _G), f32, kind="ExternalOutput")

    with TileContext(nc) as tc, tc.tile_pool(name="all", bufs=1) as pool:
        pr = pool.tile([_P, _PRW], f32, name="pr_t", tag="pr_t")
        gacc = pool.tile([_P, _G], f32, name="gacc", tag="gacc")
        tacc = pool.tile([_P, _G], f32, name="tacc", tag="tacc")
        seacc = pool.tile([_P, _G], f32, name="seacc", tag="seacc")
        lg = pool.tile([_P, _G], f32, name="lg", tag="lg")
        stmp = pool.tile([_P, _G], f32, name="stmp", tag="stmp")
        cnt = pool.tile([_P, 9, _G], f32, name="cnt", tag="cnt")
        mj = []
        for j in range(3):
            t_ = pool.tile([_P, _G, _S], bf16, name=f"mj{j}", tag=f"mj{j}")
            mj.append(t_)

        def pv(idx):  # [P,1] per-partition scalar view of params
            return pr[:, idx : idx + 1]

        nc.sync.dma_start(pr[:, :], pr_d[:, :])

        # ---- tags -> one-hot masks (bf16), then free the int tile ----
        tgt = pool.tile([_P, _G, _S], i32, name="tgt", tag="slot_tg_P1")
        nc.sync.dma_start(
            tgt[:, :, :], tg_d[:, :].rearrange("(g p) s -> p g s", g=_G)
        )
        for j in range(3):
            nc.vector.tensor_scalar(
                mj[j][:, :, :], tgt[:, :, :], float(j), None, Alu.is_equal
            )
        

        # ---- transition pair counts:  cnt[v] = sum_t mj[p][t-1]*mj[c][t] ----
        prodb = pool.tile([_P, _G, _S - 1], bf16, name="prodb", tag="slot_prod_ex")
        for v in range(9):
            p3, c3 = divmod(v, 3)
            nc.vector.tensor_tensor(
                prodb[:, :, :],
                mj[p3][:, :, 0 : _S - 1],
                mj[c3][:, :, 1:_S],
                Alu.mult,
            )
            nc.vector.tensor_reduce(cnt[:, v, :], prodb[:, :, :], Ax.X, Alu.add)
        
        nc.vector.tensor_scalar_mul(tacc[:, :], cnt[:, 0, :], pv(36))
        for v in range(1, 9):
            nc.vector.scalar_tensor_tensor(
                tacc[:, :], cnt[:, v, :], pv(36 + v), tacc[:, :], Alu.mult, Alu.add
            )

        # ---- start/end gathers ----
        nc.vector.tensor_scalar_mul(seacc[:, :], mj[0][:, :, 0], pv(45))
        for j in (1, 2):
            nc.vector.scalar_tensor_tensor(
                seacc[:, :], mj[j][:, :, 0], pv(45 + j), seacc[:, :], Alu.mult, Alu.add
            )
        for j in range(3):
            nc.vector.scalar_tensor_tensor(
                seacc[:, :],
                mj[j][:, :, _S - 1],
                pv(48 + j),
                seacc[:, :],
                Alu.mult,
                Alu.add,
            )

        nc.vector.memset(gacc[:, :], 0.0)
        nc.vector.memset(lg[:, :], 0.0)

        # ---- per-slab tiles ----
        ex = pool.tile([_P, _G, _SLAB, 3], f32, name="ex", tag="slot_prod_ex")
        q1 = _SLAB // 2
        P1 = pool.tile([_P, _G, q1, 9], f32, name="P1", tag="slot_tg_P1")
        L2 = pool.tile([_P, _G, q1 // 2, 9], f32, name="L2", tag="L2")
        L3 = pool.tile([_P, _G, q1 // 4, 9], f32, name="L3", tag="L3")
        L4 = pool.tile([_P, _G, q1 // 8, 9], f32, name="L4", tag="L4")
        L5 = pool.tile([_P, _G, q1 // 16, 9], f32, name="L5", tag="L5")
        L6 = pool.tile([_P, _G, q1 // 32, 9], f32, name="L6", tag="L6")
        deep = pool.tile([_P, _G, 4 * 8, 9], f32, name="deep", tag="deep")
        D1 = pool.tile([_P, _G, 16, 9], f32, name="D1", tag="D1")
        D2 = pool.tile([_P, _G, 8, 9], f32, name="D2", tag="D2")
        D3 = pool.tile([_P, _G, 4, 9], f32, name="D3", tag="D3")
        D4 = pool.tile([_P, _G, 2, 9], f32, name="D4", tag="D4")
        D5 = pool.tile([_P, _G, 1, 9], f32, name="D5", tag="D5")
        ts_ = pool.tile([_P, _G, q1], f32, name="ts_", tag="ts_")
        ts2 = pool.tile([_P, _G, q1], f32, name="ts2", tag="ts2")
        rm = pool.tile([_P, _G, q1 // 4], f32, name="rm", tag="rm")
        rr = pool.tile([_P, _G, q1 // 4], f32, name="rr", tag="rr")
        rlog = pool.tile([_P, _G, q1 // 4], f32, name="rlog", tag="rlog")
        sprod = pool.tile([_P, _G, _SLAB], f32, name="sprod", tag="sprod")

        def combine(Lin, Lout, qout):
            # Lout[q,(i,j)] = sum_k Lin[2q,(i,k)] * Lin[2q+1,(k,j)]
            t = ts_[:, :, :qout]
            t2 = ts2[:, :, :qout]
            for ij in range(9):
                i3, j3 = divmod(ij, 3)
                a0 = Lin[:, :, 0::2, 3 * i3 + 0]
                a1 = Lin[:, :, 0::2, 3 * i3 + 1]
                a2_ = Lin[:, :, 0::2, 3 * i3 + 2]
                b0 = Lin[:, :, 1::2, 0 + j3]
                b1 = Lin[:, :, 1::2, 3 + j3]
                b2 = Lin[:, :, 1::2, 6 + j3]
                nc.vector.tensor_tensor(t, a0, b0, Alu.mult)
                nc.vector.tensor_tensor(t2, a1, b1, Alu.mult)
                nc.vector.tensor_tensor(t, t, t2, Alu.add)
                nc.vector.tensor_tensor(t2, a2_, b2, Alu.mult)
                nc.vector.tensor_tensor(Lout[:, :, :, ij], t, t2, Alu.add)

        def renorm(L, q):
            m = rm[:, :, :q]
            r = rr[:, :, :q]
            lw = rlog[:, :, :q]
            nc.vector.tensor_reduce(m, L[:, :, :, :], Ax.X, Alu.max)
            nc.vector.reciprocal(r, m)
            rb = r.unsqueeze(3).to_broadcast([_P, _G, q, 9])
            nc.vector.tensor_tensor(L[:, :, :, :], L[:, :, :, :], rb, Alu.mult)
            nc.scalar.activation(lw, m, Act.Ln)
            nc.vector.tensor_reduce(stmp[:, :], lw, Ax.X, Alu.add)
            nc.vector.tensor_tensor(lg[:, :], lg[:, :], stmp[:, :], Alu.add)

        for sl in range(_NSLAB):
            s0 = sl * _SLAB
            nc.sync.dma_start(
                ex[:, :, :, :],
                em_d[:, s0 : s0 + _SLAB, :].rearrange(
                    "(g p) s t -> p g s t", g=_G
                ),
            )
            # gold emission gather on raw emissions
            for j in range(3):
                nc.vector.tensor_tensor(
                    sprod[:, :, :],
                    mj[j][:, :, s0 : s0 + _SLAB],
                    ex[:, :, :, j],
                    Alu.mult,
                )
                nc.vector.tensor_reduce(stmp[:, :], sprod[:, :, :], Ax.X, Alu.add)
                nc.vector.tensor_tensor(gacc[:, :], gacc[:, :], stmp[:, :], Alu.add)
            # exp in place
            nc.scalar.activation(
                ex[:, :, :, :].rearrange("p g s t -> p (g s t)"),
                ex[:, :, :, :].rearrange("p g s t -> p (g s t)"),
                Act.Exp,
            )
            # L1: P1[p,(i,j)] = E2[j] * sum_k A2[(i,j),k] E1[k]
            t = ts_[:, :, :q1]
            for ij in range(9):
                j3 = ij % 3
                nc.vector.tensor_scalar_mul(t, ex[:, :, 0::2, 0], pv(3 * ij + 0))
                nc.vector.scalar_tensor_tensor(
                    t, ex[:, :, 0::2, 1], pv(3 * ij + 1), t, Alu.mult, Alu.add
                )
                nc.vector.scalar_tensor_tensor(
                    t, ex[:, :, 0::2, 2], pv(3 * ij + 2), t, Alu.mult, Alu.add
                )
                nc.vector.tensor_tensor(
                    P1[:, :, :, ij], t, ex[:, :, 1::2, j3], Alu.mult
                )
            if sl == 0:
                # pair 0 holds virtual M0 = diag(sv*E0):
                # P1[0,(i,j)] = C0[(i,j)] * E0[i] * E1[j]
                for ij in range(9):
                    i3, j3 = divmod(ij, 3)
                    nc.vector.tensor_tensor(
                        stmp[:, :], ex[:, :, 0, i3], ex[:, :, 1, j3], Alu.mult
                    )
                    nc.vector.tensor_scalar_mul(
                        P1[:, :, 0, ij], stmp[:, :], pv(27 + ij)
                    )
            combine(P1, L2, q1 // 2)
            combine(L2, L3, q1 // 4)
            renorm(L3, q1 // 4)
            combine(L3, L4, q1 // 8)
            combine(L4, L5, q1 // 16)
            renorm(L5, q1 // 16)
            combine(L5, L6, q1 // 32)
            combine(L6, deep[:, :, sl * 8 : (sl + 1) * 8, :], q1 // 64)
            renorm(deep[:, :, sl * 8 : (sl + 1) * 8, :], q1 // 64)

        combine(deep, D1, 16)
        combine(D1, D2, 8)
        renorm(D2, 8)
        combine(D2, D3, 4)
        combine(D3, D4, 2)
        renorm(D4, 2)
        combine(D4, D5, 1)

        # z = ones^T M ev ; logZ = log(z) + lg
        colsum = D5[:, :, 0, :].rearrange("p g (i j) -> p g j i", i=3)
        t3 = ts_[:, :, 0:3]
        zt = ts2[:, :, 0:3]
        zs = rm[:, :, 0:1]
        nc.vector.tensor_reduce(t3, colsum, Ax.X, Alu.add)
        evv = pr[:, 51:57].rearrange("p (g c) -> p g c", g=_G)
        nc.vector.tensor_tensor(zt, t3, evv, Alu.mult)
        nc.vector.tensor_reduce(zs.rearrange("p g c -> p (g c)"), zt, Ax.X, Alu.add)
        lz = rr[:, :, 0:1].rearrange("p g c -> p (g c)")
        nc.scalar.activation(lz, zs.rearrange("p g c -> p (g c)"), Act.Ln)
        # loss = logZ + lg - gacc - tacc - seacc
        nc.vector.tensor_tensor(lz, lz, lg[:, :], Alu.add)
        nc.vector.tensor_tensor(lz, lz, gacc[:, :], Alu.subtract)
        nc.vector.tensor_tensor(lz, lz, tacc[:, :], Alu.subtract)
        nc.vector.tensor_tensor(lz, lz, seacc[:, :], Alu.subtract)
        nc.sync.dma_start(out_d[:, :], lz)

    nc.finalize()
    return nc


def _params_host(transitions, start_transitions, end_transitions):
    f = np.float32
    A = np.exp(transitions.astype(np.float64)).astype(f)
    sv = np.exp(start_transitions.astype(np.float64)).astype(f)
    ev = np.exp(end_transitions.astype(np.float64)).astype(f)
    A2 = np.einsum("ik,kj->ijk", A, A).reshape(27).astype(f)
    C0 = (sv[:, None] * A).reshape(9).astype(f)
    row = np.zeros(_PRW, f)
    row[0:27] = A2
    row[27:36] = C0
    row[36:45] = transitions.reshape(9).astype(f)
    row[45:48] = start_transitions.astype(f)
    row[48:51] = end_transitions.astype(f)
    row[51:57] = np.concatenate([ev, ev])
    return np.tile(row[None, :], (_P, 1))


def _fallback(emissions, transitions, start_transitions, end_transitions, tags, mask):
    # exact log-space numpy reference (only used if mask isn't all ones)
    em = emissions.astype(np.float64)
    tr = transitions.astype(np.float64)
    st = start_transitions.astype(np.float64)
    en = end_transitions.astype(np.float64)
    tg = tags.astype(np.int64)
    mk = mask.astype(np.int64)
    B, S, T = em.shape
    a = st[None, :] + em[:, 0]
    for t in range(1, S):
        m = a[:, :, None] + tr[None] + em[:, t][:, None, :]
        mx = m.max(1, keepdims=True)
        nxt = np.log(np.exp(m - mx).sum(1)) + mx[:, 0]
        a = np.where(mk[:, t : t + 1] > 0, nxt, a)
    z = a + en[None]
    mx = z.max(1, keepdims=True)
    logZ = np.log(np.exp(z - mx).sum(1)) + mx[:, 0]
    bi = np.arange(B)
    sc = st[tg[:, 0]] + em[bi, 0, tg[:, 0]]
    for t in range(1, S):
        add = tr[tg[:, t - 1], tg[:, t]] + em[bi, t, tg[:, t]]
        sc = sc + np.where(mk[:, t] > 0, add, 0.0)
    seq_lens = mk.sum(1)
    last = tg[bi, seq_lens - 1]
    sc = sc + en[last]
    return np.float32((logZ - sc).mean())


def kernel(emissions, transitions, start_transitions, end_transitions, tags, mask):
    if not np.all(mask == 1):
        return _fallback(
            emissions, transitions, start_transitions, end_transitions, tags, mask
        )
    from concourse.bass_utils import run_bass_kernel_spmd

    if "nc" not in _cache:
        _cache["nc"] = _build()
    nc = _cache["nc"]
    prh = _params_host(transitions, start_transitions, end_transitions)
    em = np.ascontiguousarray(emissions, dtype=np.float32)
    tg = np.ascontiguousarray(tags, dtype=np.int32)
    in_maps = []
    for c in range(_NC):
        b0 = c * _BL
        in_maps.append(
            {
                "em": em[b0 : b0 + _BL],
                "tg": tg[b0 : b0 + _BL],
                "pr": prh,
            }
        )
    res = run_bass_kernel_spmd(nc, in_maps, core_ids=list(range(_NC)))
    tot = np.float64(0.0)
    for c in range(_NC):
        tot += res.results[c]["out"].astype(np.float64).sum()
    return np.float32(tot / _B)

